# revision 1
# baseline (speedup 1.0000x reference)
"""Trainium2 Bass kernel for CudaTensorProduct (e3nn-style COO tensor product).

Computation: out[b, o] = sum_k cb[k] * in1[b, idx1[k]] * in2[b, idx2[k]]
  in1/in2: (16384, 32) f32, out: (16384, 1024) f32, nnz=4528.

Strategy (per core, pure data-parallel over batch, 2048 rows/core):
  - The COO table couples (i,j) input-pair columns to output columns. The
    bipartite graph decomposes into small connected components which we
    bin-pack into NG groups of (K<=128 ij-pairs, M<=128 out-cols).
  - Transpose inputs once: in12T (64, 2048) = [in1.T ; in2.T].
  - Per (group g, batch-chunk c of 512):
      R1 = E1g.T @ in12T_c   (PE, replicates in1 features to the group's K rows)
      R2 = E2g.T @ in12T_c   (PE, same for in2)
      U  = R1 * R2           (DVE elementwise -> the needed outer products)
      outT_gc = Wg.T @ U     (PE, the sparse-coefficient contraction)
    and DMA outT_gc to a (1024, 2048) transposed scratch output.
  - Host side un-transposes/un-permutes during the unshard (pure layout).

Matmuls run in float32r (TF32-like single-pass fp32) for 1 cyc/row.
"""

import os
import sys
import numpy as np

sys.path.insert(0, "/opt/trn_rl_repo")

import concourse.bass as bass
import concourse.mybir as mybir
import concourse.tile as tile
from concourse import bacc
from concourse.bass_utils import run_bass_kernel_spmd

N_CORES = 8
B = 16384
BC = B // N_CORES          # 2048 batch rows per core
D1 = 32
D2 = 32
DOUT = D1 * D2             # 1024
NG = 8                     # (K,M)<=128 groups
CHUNK = 512                # batch columns per matmul
NCHUNK = BC // CHUNK       # 4
F32 = mybir.dt.float32
F32R = mybir.dt.float32r


# ----------------------------------------------------------------------------
# Host-side table preprocessing
# ----------------------------------------------------------------------------

def _build_groups(idx1, idx2, out_idx, cb_vals):
    """Pack connected components of the (ij-col <-> out-row) graph into NG
    groups with K<=128 cols and M<=128 rows each.

    Returns (e12, w, rows_flat):
      e12: (64, NG*2*128) f32 — for group g, cols [2g*128,(2g+1)*128) hold
           E1g (rows 0:32 select i), cols [(2g+1)*128,(2g+2)*128) hold E2g
           (rows 32:64 select j).
      w:   (128, NG*128) f32 — w[:, g*128+m] holds the coefficients mapping
           group-g U rows to scratch out-row g*128+m.
      rows_flat: (NG*128,) int — scratch row r corresponds to real out col
           rows_flat[r] (-1 for padding, none expected here).
    """
    idx1 = np.asarray(idx1, np.int64)
    idx2 = np.asarray(idx2, np.int64)
    out_idx = np.asarray(out_idx, np.int64)
    cb = np.asarray(cb_vals, np.float64)
    col = idx1 * D2 + idx2

    parent = list(range(DOUT))

    def find(x):
        while parent[x] != x:
            parent[x] = parent[parent[x]]
            x = parent[x]
        return x

    col2row = {}
    for c, o in zip(col.tolist(), out_idx.tolist()):
        if c in col2row:
            ra, rb = find(col2row[c]), find(o)
            if ra != rb:
                parent[ra] = rb
        else:
            col2row[c] = o

    comp_rows, comp_cols = {}, {}
    for o in range(DOUT):
        comp_rows.setdefault(find(o), set()).add(o)
    for c, o in zip(col.tolist(), out_idx.tolist()):
        comp_cols.setdefault(find(o), set()).add(c)

    comps = [
        (sorted(comp_cols.get(k, ())), sorted(r)) for k, r in comp_rows.items()
    ]
    # drop out-rows with no terms (they are zero; none expected but be safe)
    comps = [(c, r) for c, r in comps if c]

    comps.sort(key=lambda cr: -len(cr[0]))
    bins = []
    for c, r in comps:
        for bn in bins:
            if bn["k"] + len(c) <= 128 and bn["m"] + len(r) <= 128:
                bn["cols"] += c
                bn["rows"] += r
                bn["k"] += len(c)
                bn["m"] += len(r)
                break
        else:
            bins.append({"cols": list(c), "rows": list(r), "k": len(c), "m": len(r)})
    assert len(bins) <= NG, f"packing produced {len(bins)} > {NG} groups"
    while len(bins) < NG:
        bins.append({"cols": [], "rows": [], "k": 0, "m": 0})

    # dense value map
    wmap = {}
    for c, o, v in zip(col.tolist(), out_idx.tolist(), cb.tolist()):
        wmap[(o, c)] = wmap.get((o, c), 0.0) + v

    e12 = np.zeros((64, NG * 2 * 128), np.float32)
    w = np.zeros((128, NG * 128), np.float32)
    rows_flat = np.full(NG * 128, -1, np.int64)
    for g, bn in enumerate(bins):
        cols, rows = bn["cols"], bn["rows"]
        colpos = {c: p for p, c in enumerate(cols)}
        for p, c in enumerate(cols):
            i, j = divmod(c, D2)
            e12[i, (2 * g) * 128 + p] = 1.0
            e12[32 + j, (2 * g + 1) * 128 + p] = 1.0
        for m, o in enumerate(rows):
            rows_flat[g * 128 + m] = o
        rowpos = {o: m for m, o in enumerate(rows)}
        for o in rows:
            for c in cols:
                v = wmap.get((o, c))
                if v is not None:
                    w[colpos[c], g * 128 + rowpos[o]] = np.float32(v)
    return e12, w, rows_flat


# ----------------------------------------------------------------------------
# Device program
# ----------------------------------------------------------------------------

def _build_bass():
    nc = bacc.Bacc("TRN2", target_bir_lowering=False)

    in12h = nc.dram_tensor("in12h", [BC, D1 + D2], F32, kind="ExternalInput")
    e12 = nc.dram_tensor("e12", [64, NG * 2 * 128], F32R, kind="ExternalInput")
    identw = nc.dram_tensor("identw", [128, 128], F32, kind="ExternalInput")
    wgt = nc.dram_tensor("wgt", [128, NG * 128], F32R, kind="ExternalInput")
    outT = nc.dram_tensor("outT", [DOUT, BC], F32, kind="ExternalOutput")

    NTILE = BC // 128  # 16 batch tiles for the input transpose

    with tile.TileContext(nc) as tc:
        with (
            tc.tile_pool(name="const", bufs=1) as const_pool,
            tc.tile_pool(name="inbuf", bufs=1) as in_pool,
            tc.tile_pool(name="r1sb", bufs=3) as r1_pool,
            tc.tile_pool(name="usb", bufs=5) as u_pool,
            tc.tile_pool(name="osb", bufs=6) as o_pool,
        ):
            e_sb = const_pool.tile([64, NG * 2 * 128], F32R)
            nc.sync.dma_start(out=e_sb[:], in_=e12.ap())
            w_sb = const_pool.tile([128, NG * 128], F32R)
            nc.sync.dma_start(out=w_sb[:], in_=wgt.ap())
            ident = const_pool.tile([128, 128], F32)
            nc.sync.dma_start(out=ident[:], in_=identw.ap())

            # interleaved input staging: in12[p, t, 0:32]=in1, [p, t, 32:64]=in2
            in12 = in_pool.tile([128, NTILE * 64], F32)
            in12_3d = in12[:].rearrange("p (t d) -> p t d", d=64)
            nc.sync.dma_start(
                out=in12_3d[:],
                in_=in12h.ap().rearrange("(t p) d -> p t d", p=128),
            )

            in12T = in_pool.tile([64, BC], F32R)

            # Phase 1: transpose inputs -> in12T (64, BC)
            with tc.tile_pool(name="ps_t", bufs=2, space="PSUM") as ps_t_pool:
                for tq in range(NTILE // 4):
                    ps = ps_t_pool.tile([64, 512], F32)
                    for ti in range(4):
                        t = tq * 4 + ti
                        nc.tensor.transpose(
                            ps[:, ti * 128 : (ti + 1) * 128],
                            in12_3d[:, t, :],
                            ident[:],
                        )
                    nc.scalar.copy(
                        out=in12T[:, tq * 512 : (tq + 1) * 512], in_=ps[:]
                    )

            # Phase 2: software-pipelined (chunk, group) iterations.
            # Emission lag keeps the in-order PE stream free of stalls:
            # front stage (R1/R2 + copyR + mul) runs LAG iterations ahead
            # of the back stage (main matmul + copyO + DMA).
            LAG = 2
            iters = [(c, g) for c in range(NCHUNK) for g in range(NG)]
            total = len(iters)
            pend = {}
            with (
                tc.tile_pool(name="ps_r1", bufs=2, space="PSUM") as ps_r1_pool,
                tc.tile_pool(name="ps_r2", bufs=2, space="PSUM") as ps_r2_pool,
                tc.tile_pool(name="ps_o", bufs=3, space="PSUM") as ps_o_pool,
            ):
                for it in range(total + LAG):
                    if it < total:
                        c, g = iters[it]
                        rhs = in12T[:, c * CHUNK : (c + 1) * CHUNK]
                        ps_r1 = ps_r1_pool.tile([128, CHUNK], F32)
                        nc.tensor.matmul(
                            ps_r1[:],
                            lhsT=e_sb[:, (2 * g) * 128 : (2 * g + 1) * 128],
                            rhs=rhs,
                            start=True,
                            stop=True,
                        )
                        ps_r2 = ps_r2_pool.tile([128, CHUNK], F32)
                        nc.tensor.matmul(
                            ps_r2[:],
                            lhsT=e_sb[:, (2 * g + 1) * 128 : (2 * g + 2) * 128],
                            rhs=rhs,
                            start=True,
                            stop=True,
                        )
                        r1sb = r1_pool.tile([128, CHUNK], F32)
                        nc.scalar.copy(out=r1sb[:], in_=ps_r1[:])
                        u = u_pool.tile([128, CHUNK], F32R)
                        nc.vector.tensor_mul(u[:], ps_r2[:], r1sb[:])
                        pend[it] = u
                    if it >= LAG:
                        jt = it - LAG
                        c, g = iters[jt]
                        u = pend.pop(jt)
                        ps_o = ps_o_pool.tile([128, CHUNK], F32)
                        nc.tensor.matmul(
                            ps_o[:],
                            lhsT=w_sb[:, g * 128 : (g + 1) * 128],
                            rhs=u[:],
                            start=True,
                            stop=True,
                        )
                        osb = o_pool.tile([128, CHUNK], F32)
                        # split PSUM->SBUF output copies between DVE and ACT
                        if jt % 2 == 0:
                            nc.vector.tensor_copy(osb[:], ps_o[:])
                        else:
                            nc.scalar.copy(out=osb[:], in_=ps_o[:])
                        nc.sync.dma_start(
                            out=outT.ap()[
                                g * 128 : (g + 1) * 128, c * CHUNK : (c + 1) * CHUNK
                            ],
                            in_=osb[:],
                        )
    nc.compile()
    return nc


# ----------------------------------------------------------------------------
# Entry point
# ----------------------------------------------------------------------------

_CACHE = {}


def kernel(in1, in2, cb_vals, idx1, idx2, out_idx):
    in1 = np.ascontiguousarray(np.asarray(in1, np.float32))
    in2 = np.ascontiguousarray(np.asarray(in2, np.float32))

    key = (
        np.asarray(idx1).tobytes(),
        np.asarray(idx2).tobytes(),
        np.asarray(out_idx).tobytes(),
        np.asarray(cb_vals).tobytes(),
    )
    kh = hash(key)
    if kh not in _CACHE:
        e12, w, rows_flat = _build_groups(idx1, idx2, out_idx, cb_vals)
        nc = _build_bass()
        _CACHE[kh] = (nc, e12, w, rows_flat)
    nc, e12, w, rows_flat = _CACHE[kh]

    ident = np.eye(128, dtype=np.float32)
    in12h = np.concatenate([in1, in2], axis=1)  # (B, 64)
    in_maps = []
    for core in range(N_CORES):
        sl = slice(core * BC, (core + 1) * BC)
        in_maps.append(
            {
                "in12h": np.ascontiguousarray(in12h[sl]),
                "e12": e12,
                "wgt": w,
                "identw": ident,
            }
        )

    trace = bool(int(os.environ.get("KERNEL_TRACE", "0")))
    res = run_bass_kernel_spmd(
        nc, in_maps, core_ids=list(range(N_CORES)), trace=trace
    )
    kernel.last_results = res

    out = np.empty((B, DOUT), np.float32)
    valid = rows_flat >= 0
    cols = rows_flat[valid]
    for core in range(N_CORES):
        shard = res.results[core]["outT"]  # (DOUT, BC) scratch layout
        blk = out[core * BC : (core + 1) * BC]
        blk[:, cols] = shard[valid].T
        if not valid.all():
            blk[:, ~np.isin(np.arange(DOUT), cols)] = 0.0
    return out



# revision 8
# speedup vs baseline: 1.4869x; 1.4869x over previous
"""Trainium2 Bass kernel for CudaTensorProduct (e3nn-style COO tensor product).

Computation: out[b, o] = sum_k cb[k] * in1[b, idx1[k]] * in2[b, idx2[k]]
  in1/in2: (16384, 32) f32, out: (16384, 1024) f32, nnz=4528.

Strategy (per core, pure data-parallel over batch, 2048 rows/core):
  The l-structure (ls1=ls2=[0,1,2,3]x2) factorizes: permute in1 columns into
  4 "i-sets" of 8 ({l1=0,3} and {l1=1,2} per copy); in2 columns split into
  2 "j-sets" of 16 (the two copies). Every (l1,l2,l3) coupling then lives in
  exactly one of the 8 pair-tiles q=(a,b) = iset_a x jset_b, each with
  exactly 128 (i,j) pairs AND exactly 128 output rows -> the coefficient
  matrix W is block-diagonal over q with 128x128 blocks.

  Per core (inputs host-pre-transposed to in12T (64, 2048) bf16):
    R1_a = E1a.T @ in12T   (PE; 8 i-rows each replicated 16x) -- shared by
                            both j-sets; 4 matmuls per 512-chunk total
    R2_b = E2b.T @ in12T   (PE; 16 j-rows tiled 8x) -- shared by 4 i-sets
    cast R PSUM->SBUF bf16 (ACT/GPSIMD)
    U_q  = R1_a * R2_b     (DVE scalar_tensor_tensor, all-bf16 SBUF = 4x mode)
    outT_q = W_q.T @ U_q   (PE, bf16, PSUM fp32 accum)
    cast out PSUM->SBUF bf16 (ACT/GPSIMD/DVE), DMA bf16 to HBM.

  14 weight loads + 56 matmuls of 512 cols per core; host un-permutes and
  upcasts the bf16 output to fp32 during the unshard (pure layout).
"""

import os
import sys
import numpy as np
import ml_dtypes

sys.path.insert(0, "/opt/trn_rl_repo")

import concourse.bass as bass
import concourse.mybir as mybir
import concourse.tile as tile
from concourse import bacc
from concourse.bass_utils import run_bass_kernel_spmd

N_CORES = 8
B = 16384
BC = B // N_CORES          # 2048 batch rows per core
D1 = 32
D2 = 32
DOUT = D1 * D2             # 1024
NQ = 8                     # pair-tiles (4 i-sets x 2 j-sets)
CHUNK = 512                # batch columns per matmul
NCHUNK = BC // CHUNK       # 4
F32 = mybir.dt.float32
BF16 = mybir.dt.bfloat16
MULT = mybir.AluOpType.mult

LS = [0, 1, 2, 3, 0, 1, 2, 3]


# ----------------------------------------------------------------------------
# Host-side table preprocessing
# ----------------------------------------------------------------------------

def _build_tables(idx1, idx2, out_idx, cb_vals):
    """Build the factorized layout.

    Returns (iperm, e12, w, rows_map):
      iperm: (32,) permutation of in1 columns (iset-major).
      e12: (64, 6*128) bf16 -- E1a at cols a*128.. (a=0..3), E2b at
           cols (4+b)*128.. (b=0,1); rows index in12T partitions.
      w:   (128, 8*128) bf16 -- w[p, q*128+m] = coefficient for pair p
           (p = i_local*16 + j_local) into scratch out row q*128+m.
      rows_map: (1024,) int -- scratch row -> real out column.
    """
    idx1 = np.asarray(idx1, np.int64)
    idx2 = np.asarray(idx2, np.int64)
    out_idx = np.asarray(out_idx, np.int64)
    cb = np.asarray(cb_vals, np.float64)

    offs, blocks = 0, []
    for l in LS:
        blocks.append(list(range(offs, offs + 2 * l + 1)))
        offs += 2 * l + 1
    isets = [blocks[0] + blocks[3], blocks[1] + blocks[2],
             blocks[4] + blocks[7], blocks[5] + blocks[6]]
    jsets = [list(range(16)), list(range(16, 32))]
    imap = {c: (a, il) for a, s in enumerate(isets) for il, c in enumerate(s)}
    jmap = {c: (b, jl) for b, s in enumerate(jsets) for jl, c in enumerate(s)}

    out_q = {}
    for k in range(len(cb)):
        a, _ = imap[int(idx1[k])]
        b, _ = jmap[int(idx2[k])]
        q = a * 2 + b
        o = int(out_idx[k])
        assert out_q.setdefault(o, q) == q, "coupling crosses pair-tiles"
    rows_map = np.zeros(NQ * 128, np.int64)
    out_local = {}
    for q in range(NQ):
        outs = sorted(o for o, qq in out_q.items() if qq == q)
        assert len(outs) == 128, (q, len(outs))
        for m, o in enumerate(outs):
            out_local[o] = m
            rows_map[q * 128 + m] = o

    e12 = np.zeros((64, 6 * 128), np.float32)
    for a in range(4):
        for p in range(128):
            e12[a * 8 + p // 16, a * 128 + p] = 1.0
    for b in range(2):
        for p in range(128):
            e12[32 + b * 16 + p % 16, (4 + b) * 128 + p] = 1.0

    w = np.zeros((128, NQ * 128), np.float64)
    for k in range(len(cb)):
        a, il = imap[int(idx1[k])]
        b, jl = jmap[int(idx2[k])]
        q = a * 2 + b
        p = il * 16 + jl
        m = out_local[int(out_idx[k])]
        w[p, q * 128 + m] += cb[k]

    iperm = np.concatenate([np.asarray(s) for s in isets])
    bf = ml_dtypes.bfloat16
    return iperm, e12.astype(bf), w.astype(np.float32).astype(bf), rows_map


# ----------------------------------------------------------------------------
# Device program
# ----------------------------------------------------------------------------

def _build_bass():
    nc = bacc.Bacc("TRN2", target_bir_lowering=False)

    in12h = nc.dram_tensor("in12h", [64, BC], BF16, kind="ExternalInput")
    e12 = nc.dram_tensor("e12", [64, 6 * 128], BF16, kind="ExternalInput")
    wgt = nc.dram_tensor("wgt", [128, NQ * 128], BF16, kind="ExternalInput")
    outT = nc.dram_tensor("outT", [NQ * 128, BC], BF16, kind="ExternalOutput")

    # emission plan: R slots 0-3 = R1 (isets), 4-5 = R2 (jsets)
    rplan = [[0, 4], [1, 5], [2], [3]]
    qplan = [(0, 0), (1, 1), (2, 0), (3, 1), (0, 1), (1, 0), (2, 1), (3, 0)]
    # mul for (a,b) becomes ready after rplan step max(step(a), step(b));
    # GPSIMD (slow but otherwise idle; SBUF-only) takes the muls with the
    # most slack, emitted at their earliest-ready step.
    mul_ready = {0: [(0, 0)], 1: [(0, 1), (1, 0), (1, 1)],
                 2: [(2, 0), (2, 1)], 3: [(3, 1), (3, 0)]}
    gps_muls = {(0, 1), (1, 0), (2, 1)}
    LAG = 2

    with tile.TileContext(nc) as tc:
        with (
            tc.tile_pool(name="const", bufs=1) as const_pool,
            tc.tile_pool(name="work", bufs=1) as work_pool,
            tc.tile_pool(name="ps_r", bufs=2, space="PSUM") as ps_r_pool,
            tc.tile_pool(name="ps_o", bufs=2, space="PSUM") as ps_o_pool,
        ):
            e_sb = const_pool.tile([64, 6 * 128], BF16)
            nc.sync.dma_start(out=e_sb[:], in_=e12.ap())
            w_sb = const_pool.tile([128, NQ * 128], BF16)
            nc.sync.dma_start(out=w_sb[:], in_=wgt.ap())
            x_sb = work_pool.tile([64, BC], BF16)
            nc.sync.dma_start(out=x_sb[:], in_=in12h.ap())

            r_sb = work_pool.tile([128, 6 * BC], BF16)
            u_sb = work_pool.tile([128, NQ * BC], BF16)
            osb = work_pool.tile([128, NQ * BC], BF16)

            # PSUM->SBUF bf16 casts: ACT-heavy, DVE takes the rest (GPSIMD
            # cannot access PSUM). ~16 ACT / 12 DVE of 28 total.
            cast_engines = [nc.scalar, nc.scalar, nc.vector, nc.scalar,
                            nc.vector, nc.scalar, nc.vector]
            cast_i = [0]

            def emit_cast(dst, ps):
                eng = cast_engines[cast_i[0] % len(cast_engines)]
                cast_i[0] += 1
                if eng is nc.scalar:
                    eng.copy(out=dst, in_=ps[:])
                else:
                    eng.tensor_copy(dst, ps[:])

            def emit_mul(a, b):
                q = a * 2 + b
                out_ap = u_sb[:, q * BC : (q + 1) * BC]
                in0 = r_sb[:, a * BC : (a + 1) * BC]
                in1 = r_sb[:, (4 + b) * BC : (5 + b) * BC]
                if (a, b) in gps_muls:
                    # Pool engine: only plain TensorTensor is supported
                    nc.gpsimd.tensor_mul(out_ap, in0, in1)
                else:
                    # DVE: TensorScalarPtr form runs in 4x mode on bf16 SBUF
                    nc.vector.scalar_tensor_tensor(
                        out=out_ap, in0=in0, scalar=1.0, in1=in1,
                        op0=MULT, op1=MULT,
                    )

            for it in range(NQ + LAG):
                # R-matmuls + casts (1024-wide casts, 2 matmuls per psum tile)
                if it < len(rplan):
                    for slot in rplan[it]:
                        for h in range(2):  # halves of BC: 2 chunks each
                            ps = ps_r_pool.tile([128, 2 * CHUNK], F32)
                            for ci in range(2):
                                c = h * 2 + ci
                                nc.tensor.matmul(
                                    ps[:, ci * CHUNK : (ci + 1) * CHUNK],
                                    lhsT=e_sb[:, slot * 128 : (slot + 1) * 128],
                                    rhs=x_sb[:, c * CHUNK : (c + 1) * CHUNK],
                                    start=True,
                                    stop=True,
                                )
                            emit_cast(
                                r_sb[:, slot * BC + h * 1024 : slot * BC + (h + 1) * 1024],
                                ps,
                            )
                    # U muls at earliest-ready (4x-mode on DVE, eff-0.6 on GPS)
                    for a, b in mul_ready.get(it, ()):
                        emit_mul(a, b)
                # main matmuls + out casts + DMA (lagged)
                if it >= LAG:
                    a, b = qplan[it - LAG]
                    q = a * 2 + b
                    for h in range(2):
                        ps = ps_o_pool.tile([128, 2 * CHUNK], F32)
                        for ci in range(2):
                            c = h * 2 + ci
                            nc.tensor.matmul(
                                ps[:, ci * CHUNK : (ci + 1) * CHUNK],
                                lhsT=w_sb[:, q * 128 : (q + 1) * 128],
                                rhs=u_sb[:, q * BC + c * CHUNK : q * BC + (c + 1) * CHUNK],
                                start=True,
                                stop=True,
                            )
                        emit_cast(
                            osb[:, q * BC + h * 1024 : q * BC + (h + 1) * 1024],
                            ps,
                        )
                    nc.sync.dma_start(
                        out=outT.ap()[q * 128 : (q + 1) * 128, :],
                        in_=osb[:, q * BC : (q + 1) * BC],
                    )
    nc.compile()
    return nc


# ----------------------------------------------------------------------------
# Entry point
# ----------------------------------------------------------------------------

_CACHE = {}


def kernel(in1, in2, cb_vals, idx1, idx2, out_idx):
    in1 = np.ascontiguousarray(np.asarray(in1, np.float32))
    in2 = np.ascontiguousarray(np.asarray(in2, np.float32))

    key = (
        np.asarray(idx1).tobytes(),
        np.asarray(idx2).tobytes(),
        np.asarray(out_idx).tobytes(),
        np.asarray(cb_vals).tobytes(),
    )
    kh = hash(key)
    if kh not in _CACHE:
        iperm, e12, w, rows_map = _build_tables(idx1, idx2, out_idx, cb_vals)
        nc = _build_bass()
        _CACHE[kh] = (nc, iperm, e12, w, rows_map)
    nc, iperm, e12, w, rows_map = _CACHE[kh]

    bf = ml_dtypes.bfloat16
    in1p = in1[:, iperm]
    in_maps = []
    for core in range(N_CORES):
        sl = slice(core * BC, (core + 1) * BC)
        in12h = np.ascontiguousarray(
            np.concatenate([in1p[sl], in2[sl]], axis=1).T.astype(bf)
        )  # (64, BC)
        in_maps.append({"in12h": in12h, "e12": e12, "wgt": w})

    trace = bool(int(os.environ.get("KERNEL_TRACE", "0")))
    res = run_bass_kernel_spmd(
        nc, in_maps, core_ids=list(range(N_CORES)), trace=trace
    )
    kernel.last_results = res

    out = np.empty((B, DOUT), np.float32)
    for core in range(N_CORES):
        shard = res.results[core]["outT"]  # (1024, BC) bf16 scratch layout
        out[core * BC : (core + 1) * BC][:, rows_map] = (
            np.asarray(shard).astype(np.float32).T
        )
    return out


# revision 12
# speedup vs baseline: 1.6400x; 1.1030x over previous
"""Trainium2 Bass kernel for CudaTensorProduct (e3nn-style COO tensor product).

Computation: out[b, o] = sum_k cb[k] * in1[b, idx1[k]] * in2[b, idx2[k]]
  in1/in2: (16384, 32) f32, out: (16384, 1024) f32, nnz=4528.

Strategy (per core, pure data-parallel over batch, 2048 rows/core):
  The l-structure (ls1=ls2=[0,1,2,3]x2) factorizes: permute in1 columns into
  4 "i-sets" of 8 ({l1=0,3} and {l1=1,2} per copy); in2 columns split into
  2 "j-sets" of 16 (the two copies). Every (l1,l2,l3) coupling then lives in
  exactly one of the 8 pair-tiles q=(a,b) = iset_a x jset_b, each with
  exactly 128 (i,j) pairs AND exactly 128 output rows -> the coefficient
  matrix W is block-diagonal over q with 128x128 blocks.

  Per core (inputs host-pre-transposed to in12T (64, 2048) bf16):
    R1_a = E1a.T @ in12T   (PE; 8 i-rows each replicated 16x) -- shared by
                            both j-sets; 4 matmuls per 512-chunk total
    R2_b = E2b.T @ in12T   (PE; 16 j-rows tiled 8x) -- shared by 4 i-sets
    cast R PSUM->SBUF bf16 (ACT/GPSIMD)
    U_q  = R1_a * R2_b     (DVE scalar_tensor_tensor, all-bf16 SBUF = 4x mode)
    outT_q = W_q.T @ U_q   (PE, bf16, PSUM fp32 accum)
    cast out PSUM->SBUF bf16 (ACT/GPSIMD/DVE), DMA bf16 to HBM.

  14 weight loads + 56 matmuls of 512 cols per core; host un-permutes and
  upcasts the bf16 output to fp32 during the unshard (pure layout).
"""

import os
import sys
import numpy as np
import ml_dtypes

sys.path.insert(0, "/opt/trn_rl_repo")

import concourse.bass as bass
import concourse.mybir as mybir
import concourse.tile as tile
from concourse import bacc
from concourse.bass_utils import run_bass_kernel_spmd

N_CORES = 8
B = 16384
BC = B // N_CORES          # 2048 batch rows per core
D1 = 32
D2 = 32
DOUT = D1 * D2             # 1024
NQ = 8                     # pair-tiles (4 i-sets x 2 j-sets)
CHUNK = 512                # batch columns per matmul
NCHUNK = BC // CHUNK       # 4
F32 = mybir.dt.float32
BF16 = mybir.dt.bfloat16
MULT = mybir.AluOpType.mult

LS = [0, 1, 2, 3, 0, 1, 2, 3]


# ----------------------------------------------------------------------------
# Host-side table preprocessing
# ----------------------------------------------------------------------------

def _build_tables(idx1, idx2, out_idx, cb_vals):
    """Build the factorized layout.

    Returns (iperm, e12, w, rows_map):
      iperm: (32,) permutation of in1 columns (iset-major).
      e12: (64, 6*128) bf16 -- E1a at cols a*128.. (a=0..3), E2b at
           cols (4+b)*128.. (b=0,1); rows index in12T partitions.
      w:   (128, 8*128) bf16 -- w[p, q*128+m] = coefficient for pair p
           (p = i_local*16 + j_local) into scratch out row q*128+m.
      rows_map: (1024,) int -- scratch row -> real out column.
    """
    idx1 = np.asarray(idx1, np.int64)
    idx2 = np.asarray(idx2, np.int64)
    out_idx = np.asarray(out_idx, np.int64)
    cb = np.asarray(cb_vals, np.float64)

    offs, blocks = 0, []
    for l in LS:
        blocks.append(list(range(offs, offs + 2 * l + 1)))
        offs += 2 * l + 1
    isets = [blocks[0] + blocks[3], blocks[1] + blocks[2],
             blocks[4] + blocks[7], blocks[5] + blocks[6]]
    jsets = [list(range(16)), list(range(16, 32))]
    imap = {c: (a, il) for a, s in enumerate(isets) for il, c in enumerate(s)}
    jmap = {c: (b, jl) for b, s in enumerate(jsets) for jl, c in enumerate(s)}

    out_q = {}
    for k in range(len(cb)):
        a, _ = imap[int(idx1[k])]
        b, _ = jmap[int(idx2[k])]
        q = a * 2 + b
        o = int(out_idx[k])
        assert out_q.setdefault(o, q) == q, "coupling crosses pair-tiles"
    rows_map = np.zeros(NQ * 128, np.int64)
    out_local = {}
    for q in range(NQ):
        outs = sorted(o for o, qq in out_q.items() if qq == q)
        assert len(outs) == 128, (q, len(outs))
        for m, o in enumerate(outs):
            out_local[o] = m
            rows_map[q * 128 + m] = o

    e12 = np.zeros((64, 6 * 128), np.float32)
    for a in range(4):
        for p in range(128):
            e12[a * 8 + p // 16, a * 128 + p] = 1.0
    for b in range(2):
        for p in range(128):
            e12[32 + b * 16 + p % 16, (4 + b) * 128 + p] = 1.0

    w = np.zeros((128, NQ * 128), np.float64)
    for k in range(len(cb)):
        a, il = imap[int(idx1[k])]
        b, jl = jmap[int(idx2[k])]
        q = a * 2 + b
        p = il * 16 + jl
        m = out_local[int(out_idx[k])]
        w[p, q * 128 + m] += cb[k]

    iperm = np.concatenate([np.asarray(s) for s in isets])
    bf = ml_dtypes.bfloat16
    return iperm, e12.astype(bf), w.astype(np.float32).astype(bf), rows_map


# ----------------------------------------------------------------------------
# Device program
# ----------------------------------------------------------------------------

def _build_bass():
    nc = bacc.Bacc("TRN2", target_bir_lowering=False)

    in12h = nc.dram_tensor("in12h", [64, BC], BF16, kind="ExternalInput")
    e12 = nc.dram_tensor("e12", [64, 6 * 128], BF16, kind="ExternalInput")
    wgt = nc.dram_tensor("wgt", [128, NQ * 128], BF16, kind="ExternalInput")
    outT = nc.dram_tensor("outT", [NQ * 128, BC], BF16, kind="ExternalOutput")

    # emission plan: R slots 0-3 = R1 (isets), 4-5 = R2 (jsets)
    rplan = [[0, 4], [1, 5], [2], [3]]
    qplan = [(0, 0), (1, 1), (2, 0), (3, 1), (0, 1), (1, 0), (2, 1), (3, 0)]
    # mul for (a,b) becomes ready after rplan step max(step(a), step(b));
    # GPSIMD (slow but otherwise idle; SBUF-only) takes the muls with the
    # most slack, emitted at their earliest-ready step.
    mul_ready = {0: [(0, 0)], 1: [(0, 1), (1, 0), (1, 1)],
                 2: [(2, 0), (2, 1)], 3: [(3, 1), (3, 0)]}
    gps_muls = {(0, 1), (1, 0)}
    LAG = 2

    with tile.TileContext(nc) as tc:
        with (
            tc.tile_pool(name="const", bufs=1) as const_pool,
            tc.tile_pool(name="work", bufs=1) as work_pool,
            tc.tile_pool(name="ps_r", bufs=2, space="PSUM") as ps_r_pool,
            tc.tile_pool(name="ps_o", bufs=2, space="PSUM") as ps_o_pool,
        ):
            x_sb = work_pool.tile([64, BC], BF16)
            nc.sync.dma_start(out=x_sb[:], in_=in12h.ap())
            e_sb = const_pool.tile([64, 6 * 128], BF16)
            nc.sync.dma_start(out=e_sb[:], in_=e12.ap())
            w_sb = const_pool.tile([128, NQ * 128], BF16)
            nc.sync.dma_start(out=w_sb[:], in_=wgt.ap())

            r_sb = work_pool.tile([128, 6 * BC], BF16)
            u_sb = work_pool.tile([128, NQ * BC], BF16)
            osb = work_pool.tile([128, NQ * BC], BF16)

            # PSUM->SBUF bf16 casts: ACT-heavy, DVE takes the rest (GPSIMD
            # cannot access PSUM). ~17 ACT / 11 DVE of 28 total.
            cast_engines = [nc.scalar, nc.scalar, nc.vector, nc.scalar,
                            nc.vector, nc.scalar, nc.scalar, nc.vector]
            cast_i = [0]

            def emit_cast(dst, ps):
                eng = cast_engines[cast_i[0] % len(cast_engines)]
                cast_i[0] += 1
                if eng is nc.scalar:
                    eng.copy(out=dst, in_=ps[:])
                else:
                    eng.tensor_copy(dst, ps[:])

            def emit_mul(a, b):
                q = a * 2 + b
                out_ap = u_sb[:, q * BC : (q + 1) * BC]
                in0 = r_sb[:, a * BC : (a + 1) * BC]
                in1 = r_sb[:, (4 + b) * BC : (5 + b) * BC]
                if (a, b) in gps_muls:
                    # Pool engine: only plain TensorTensor is supported
                    nc.gpsimd.tensor_mul(out_ap, in0, in1)
                else:
                    # DVE plain TensorTensor: 2x_1p mode on packed bf16
                    nc.vector.tensor_mul(out_ap, in0, in1)

            for it in range(NQ + LAG):
                # R-matmuls + casts (1024-wide casts, 2 matmuls per psum tile)
                if it < len(rplan):
                    for slot in rplan[it]:
                        for h in range(2):  # halves of BC: 2 chunks each
                            ps = ps_r_pool.tile([128, 2 * CHUNK], F32)
                            for ci in range(2):
                                c = h * 2 + ci
                                nc.tensor.matmul(
                                    ps[:, ci * CHUNK : (ci + 1) * CHUNK],
                                    lhsT=e_sb[:, slot * 128 : (slot + 1) * 128],
                                    rhs=x_sb[:, c * CHUNK : (c + 1) * CHUNK],
                                    start=True,
                                    stop=True,
                                )
                            emit_cast(
                                r_sb[:, slot * BC + h * 1024 : slot * BC + (h + 1) * 1024],
                                ps,
                            )
                    # U muls at earliest-ready (4x-mode on DVE, eff-0.6 on GPS)
                    for a, b in mul_ready.get(it, ()):
                        emit_mul(a, b)
                # main matmuls + out casts + DMA (lagged)
                if it >= LAG:
                    a, b = qplan[it - LAG]
                    q = a * 2 + b
                    for h in range(2):
                        ps = ps_o_pool.tile([128, 2 * CHUNK], F32)
                        for ci in range(2):
                            c = h * 2 + ci
                            nc.tensor.matmul(
                                ps[:, ci * CHUNK : (ci + 1) * CHUNK],
                                lhsT=w_sb[:, q * 128 : (q + 1) * 128],
                                rhs=u_sb[:, q * BC + c * CHUNK : q * BC + (c + 1) * CHUNK],
                                start=True,
                                stop=True,
                            )
                        emit_cast(
                            osb[:, q * BC + h * 1024 : q * BC + (h + 1) * 1024],
                            ps,
                        )
                    nc.sync.dma_start(
                        out=outT.ap()[q * 128 : (q + 1) * 128, :],
                        in_=osb[:, q * BC : (q + 1) * BC],
                    )
    nc.compile()
    return nc


# ----------------------------------------------------------------------------
# Entry point
# ----------------------------------------------------------------------------

_CACHE = {}


def kernel(in1, in2, cb_vals, idx1, idx2, out_idx):
    in1 = np.ascontiguousarray(np.asarray(in1, np.float32))
    in2 = np.ascontiguousarray(np.asarray(in2, np.float32))

    key = (
        np.asarray(idx1).tobytes(),
        np.asarray(idx2).tobytes(),
        np.asarray(out_idx).tobytes(),
        np.asarray(cb_vals).tobytes(),
    )
    kh = hash(key)
    if kh not in _CACHE:
        iperm, e12, w, rows_map = _build_tables(idx1, idx2, out_idx, cb_vals)
        nc = _build_bass()
        _CACHE[kh] = (nc, iperm, e12, w, rows_map)
    nc, iperm, e12, w, rows_map = _CACHE[kh]

    bf = ml_dtypes.bfloat16
    in1p = in1[:, iperm]
    in_maps = []
    for core in range(N_CORES):
        sl = slice(core * BC, (core + 1) * BC)
        in12h = np.ascontiguousarray(
            np.concatenate([in1p[sl], in2[sl]], axis=1).T.astype(bf)
        )  # (64, BC)
        in_maps.append({"in12h": in12h, "e12": e12, "wgt": w})

    trace = bool(int(os.environ.get("KERNEL_TRACE", "0")))
    res = run_bass_kernel_spmd(
        nc, in_maps, core_ids=list(range(N_CORES)), trace=trace
    )
    kernel.last_results = res

    out = np.empty((B, DOUT), np.float32)
    for core in range(N_CORES):
        shard = res.results[core]["outT"]  # (1024, BC) bf16 scratch layout
        out[core * BC : (core + 1) * BC][:, rows_map] = (
            np.asarray(shard).astype(np.float32).T
        )
    return out


# revision 13
# speedup vs baseline: 1.6786x; 1.0235x over previous
"""Trainium2 Bass kernel for CudaTensorProduct (e3nn-style COO tensor product).

Computation: out[b, o] = sum_k cb[k] * in1[b, idx1[k]] * in2[b, idx2[k]]
  in1/in2: (16384, 32) f32, out: (16384, 1024) f32, nnz=4528.

Strategy (per core, pure data-parallel over batch, 2048 rows/core):
  The l-structure (ls1=ls2=[0,1,2,3]x2) factorizes: permute in1 columns into
  4 "i-sets" of 8 ({l1=0,3} and {l1=1,2} per copy); in2 columns split into
  2 "j-sets" of 16 (the two copies). Every (l1,l2,l3) coupling then lives in
  exactly one of the 8 pair-tiles q=(a,b) = iset_a x jset_b, each with
  exactly 128 (i,j) pairs AND exactly 128 output rows -> the coefficient
  matrix W is block-diagonal over q with 128x128 blocks.

  Per core (inputs host-pre-transposed to in12T (64, 2048) bf16):
    R1_a = E1a.T @ in12T   (PE; 8 i-rows each replicated 16x) -- shared by
                            both j-sets; 4 matmuls per 512-chunk total
    R2_b = E2b.T @ in12T   (PE; 16 j-rows tiled 8x) -- shared by 4 i-sets
    cast R PSUM->SBUF bf16 (ACT/GPSIMD)
    U_q  = R1_a * R2_b     (DVE scalar_tensor_tensor, all-bf16 SBUF = 4x mode)
    outT_q = W_q.T @ U_q   (PE, bf16, PSUM fp32 accum)
    cast out PSUM->SBUF bf16 (ACT/GPSIMD/DVE), DMA bf16 to HBM.

  14 weight loads + 56 matmuls of 512 cols per core; host un-permutes and
  upcasts the bf16 output to fp32 during the unshard (pure layout).
"""

import os
import sys
import numpy as np
import ml_dtypes

sys.path.insert(0, "/opt/trn_rl_repo")

import concourse.bass as bass
import concourse.mybir as mybir
import concourse.tile as tile
from concourse import bacc
from concourse.bass_utils import run_bass_kernel_spmd

N_CORES = 8
B = 16384
BC = B // N_CORES          # 2048 batch rows per core
D1 = 32
D2 = 32
DOUT = D1 * D2             # 1024
NQ = 8                     # pair-tiles (4 i-sets x 2 j-sets)
CHUNK = 512                # batch columns per matmul
NCHUNK = BC // CHUNK       # 4
F32 = mybir.dt.float32
BF16 = mybir.dt.bfloat16
MULT = mybir.AluOpType.mult

LS = [0, 1, 2, 3, 0, 1, 2, 3]


# ----------------------------------------------------------------------------
# Host-side table preprocessing
# ----------------------------------------------------------------------------

def _build_tables(idx1, idx2, out_idx, cb_vals):
    """Build the factorized layout.

    Returns (iperm, e12, w, rows_map):
      iperm: (32,) permutation of in1 columns (iset-major).
      e12: (64, 6*128) bf16 -- E1a at cols a*128.. (a=0..3), E2b at
           cols (4+b)*128.. (b=0,1); rows index in12T partitions.
      w:   (128, 8*128) bf16 -- w[p, q*128+m] = coefficient for pair p
           (p = i_local*16 + j_local) into scratch out row q*128+m.
      rows_map: (1024,) int -- scratch row -> real out column.
    """
    idx1 = np.asarray(idx1, np.int64)
    idx2 = np.asarray(idx2, np.int64)
    out_idx = np.asarray(out_idx, np.int64)
    cb = np.asarray(cb_vals, np.float64)

    offs, blocks = 0, []
    for l in LS:
        blocks.append(list(range(offs, offs + 2 * l + 1)))
        offs += 2 * l + 1
    isets = [blocks[0] + blocks[3], blocks[1] + blocks[2],
             blocks[4] + blocks[7], blocks[5] + blocks[6]]
    jsets = [list(range(16)), list(range(16, 32))]
    imap = {c: (a, il) for a, s in enumerate(isets) for il, c in enumerate(s)}
    jmap = {c: (b, jl) for b, s in enumerate(jsets) for jl, c in enumerate(s)}

    out_q = {}
    for k in range(len(cb)):
        a, _ = imap[int(idx1[k])]
        b, _ = jmap[int(idx2[k])]
        q = a * 2 + b
        o = int(out_idx[k])
        assert out_q.setdefault(o, q) == q, "coupling crosses pair-tiles"
    rows_map = np.zeros(NQ * 128, np.int64)
    out_local = {}
    for q in range(NQ):
        outs = sorted(o for o, qq in out_q.items() if qq == q)
        assert len(outs) == 128, (q, len(outs))
        for m, o in enumerate(outs):
            out_local[o] = m
            rows_map[q * 128 + m] = o

    e12 = np.zeros((64, 6 * 128), np.float32)
    for a in range(4):
        for p in range(128):
            e12[a * 8 + p // 16, a * 128 + p] = 1.0
    for b in range(2):
        for p in range(128):
            e12[32 + b * 16 + p % 16, (4 + b) * 128 + p] = 1.0

    w = np.zeros((128, NQ * 128), np.float64)
    for k in range(len(cb)):
        a, il = imap[int(idx1[k])]
        b, jl = jmap[int(idx2[k])]
        q = a * 2 + b
        p = il * 16 + jl
        m = out_local[int(out_idx[k])]
        w[p, q * 128 + m] += cb[k]

    iperm = np.concatenate([np.asarray(s) for s in isets])
    bf = ml_dtypes.bfloat16
    return iperm, e12.astype(bf), w.astype(np.float32).astype(bf), rows_map


# ----------------------------------------------------------------------------
# Device program
# ----------------------------------------------------------------------------

def _build_bass():
    nc = bacc.Bacc("TRN2", target_bir_lowering=False)

    in12h = nc.dram_tensor("in12h", [64, BC], BF16, kind="ExternalInput")
    e12 = nc.dram_tensor("e12", [64, 6 * 128], BF16, kind="ExternalInput")
    wgt = nc.dram_tensor("wgt", [128, NQ * 128], BF16, kind="ExternalInput")
    outT = nc.dram_tensor("outT", [NQ * 128, BC], BF16, kind="ExternalOutput")

    # R slots 0-3 = R1 (isets), 4-5 = R2 (jsets); emission order interleaves
    # the slot pairs so muls unblock early.
    slot_order = [0, 4, 1, 5, 2, 3]
    qplan = [(0, 0), (1, 1), (2, 0), (3, 1), (0, 1), (1, 0), (2, 1), (3, 0)]
    # mul (a,b) is ready once slots a and 4+b are cast; emitted right after
    # the unblocking slot. GPSIMD (slow, SBUF-only, otherwise idle) takes
    # late-consumed muls.
    mul_after_slot = {4: [(0, 0)], 5: [(0, 1)], 1: [(1, 1), (1, 0)],
                     2: [(2, 0), (2, 1)], 3: [(3, 1), (3, 0)]}
    gps_muls = {(0, 1), (1, 0), (2, 1)}

    with tile.TileContext(nc) as tc:
        with (
            tc.tile_pool(name="const", bufs=1) as const_pool,
            tc.tile_pool(name="work", bufs=1) as work_pool,
            tc.tile_pool(name="ps_r", bufs=2, space="PSUM") as ps_r_pool,
            tc.tile_pool(name="ps_o", bufs=2, space="PSUM") as ps_o_pool,
        ):
            e_sb = const_pool.tile([64, 6 * 128], BF16)
            nc.sync.dma_start(out=e_sb[:], in_=e12.ap())
            x_sb = work_pool.tile([64, BC], BF16)
            # split the input DMA so the first R-matmuls start sooner
            nc.sync.dma_start(out=x_sb[:, : BC // 2], in_=in12h.ap()[:, : BC // 2])
            nc.sync.dma_start(out=x_sb[:, BC // 2 :], in_=in12h.ap()[:, BC // 2 :])
            w_sb = const_pool.tile([128, NQ * 128], BF16)
            nc.sync.dma_start(out=w_sb[:], in_=wgt.ap())

            r_sb = work_pool.tile([128, 6 * BC], BF16)
            u_sb = work_pool.tile([128, NQ * BC], BF16)
            osb = work_pool.tile([128, NQ * BC], BF16)

            # PSUM->SBUF bf16 casts (GPSIMD cannot access PSUM):
            # strict ACT/DVE alternation in the R phase keeps PE gap-free;
            # ACT-heavy in the main phase since DVE also owns the muls.
            r_cast_engines = [nc.scalar, nc.vector]
            o_cast_engines = [nc.scalar, nc.scalar, nc.vector]
            cast_i = [0]

            def emit_cast(engines, dst, ps):
                eng = engines[cast_i[0] % len(engines)]
                cast_i[0] += 1
                if eng is nc.scalar:
                    eng.copy(out=dst, in_=ps[:])
                else:
                    eng.tensor_copy(dst, ps[:])

            def emit_mul(a, b):
                q = a * 2 + b
                out_ap = u_sb[:, q * BC : (q + 1) * BC]
                in0 = r_sb[:, a * BC : (a + 1) * BC]
                in1 = r_sb[:, (4 + b) * BC : (5 + b) * BC]
                if (a, b) in gps_muls:
                    # Pool engine: only plain TensorTensor is supported
                    nc.gpsimd.tensor_mul(out_ap, in0, in1)
                else:
                    # DVE plain TensorTensor: 2x_1p mode on packed bf16
                    nc.vector.tensor_mul(out_ap, in0, in1)

            # Phase A: all 24 R-matmuls back-to-back (PE p-state ramp),
            # casts alternating ACT/DVE, muls at earliest-ready.
            for slot in slot_order:
                for h in range(2):  # halves of BC: 2 chunks each
                    ps = ps_r_pool.tile([128, 2 * CHUNK], F32)
                    for ci in range(2):
                        c = h * 2 + ci
                        nc.tensor.matmul(
                            ps[:, ci * CHUNK : (ci + 1) * CHUNK],
                            lhsT=e_sb[:, slot * 128 : (slot + 1) * 128],
                            rhs=x_sb[:, c * CHUNK : (c + 1) * CHUNK],
                            start=True,
                            stop=True,
                        )
                    emit_cast(
                        r_cast_engines,
                        r_sb[:, slot * BC + h * 1024 : slot * BC + (h + 1) * 1024],
                        ps,
                    )
                for a, b in mul_after_slot.get(slot, ()):
                    emit_mul(a, b)

            # Phase B: mains + out casts + per-half DMA
            cast_i[0] = 0
            for a, b in qplan:
                q = a * 2 + b
                for h in range(2):
                    ps = ps_o_pool.tile([128, 2 * CHUNK], F32)
                    for ci in range(2):
                        c = h * 2 + ci
                        nc.tensor.matmul(
                            ps[:, ci * CHUNK : (ci + 1) * CHUNK],
                            lhsT=w_sb[:, q * 128 : (q + 1) * 128],
                            rhs=u_sb[:, q * BC + c * CHUNK : q * BC + (c + 1) * CHUNK],
                            start=True,
                            stop=True,
                        )
                    emit_cast(
                        o_cast_engines,
                        osb[:, q * BC + h * 1024 : q * BC + (h + 1) * 1024],
                        ps,
                    )
                    nc.sync.dma_start(
                        out=outT.ap()[
                            q * 128 : (q + 1) * 128, h * 1024 : (h + 1) * 1024
                        ],
                        in_=osb[:, q * BC + h * 1024 : q * BC + (h + 1) * 1024],
                    )
    nc.compile()
    return nc


# ----------------------------------------------------------------------------
# Entry point
# ----------------------------------------------------------------------------

_CACHE = {}


def kernel(in1, in2, cb_vals, idx1, idx2, out_idx):
    in1 = np.ascontiguousarray(np.asarray(in1, np.float32))
    in2 = np.ascontiguousarray(np.asarray(in2, np.float32))

    key = (
        np.asarray(idx1).tobytes(),
        np.asarray(idx2).tobytes(),
        np.asarray(out_idx).tobytes(),
        np.asarray(cb_vals).tobytes(),
    )
    kh = hash(key)
    if kh not in _CACHE:
        iperm, e12, w, rows_map = _build_tables(idx1, idx2, out_idx, cb_vals)
        nc = _build_bass()
        _CACHE[kh] = (nc, iperm, e12, w, rows_map)
    nc, iperm, e12, w, rows_map = _CACHE[kh]

    bf = ml_dtypes.bfloat16
    in1p = in1[:, iperm]
    in_maps = []
    for core in range(N_CORES):
        sl = slice(core * BC, (core + 1) * BC)
        in12h = np.ascontiguousarray(
            np.concatenate([in1p[sl], in2[sl]], axis=1).T.astype(bf)
        )  # (64, BC)
        in_maps.append({"in12h": in12h, "e12": e12, "wgt": w})

    trace = bool(int(os.environ.get("KERNEL_TRACE", "0")))
    res = run_bass_kernel_spmd(
        nc, in_maps, core_ids=list(range(N_CORES)), trace=trace
    )
    kernel.last_results = res

    out = np.empty((B, DOUT), np.float32)
    for core in range(N_CORES):
        shard = res.results[core]["outT"]  # (1024, BC) bf16 scratch layout
        out[core * BC : (core + 1) * BC][:, rows_map] = (
            np.asarray(shard).astype(np.float32).T
        )
    return out


# revision 15
# speedup vs baseline: 1.7784x; 1.0595x over previous
"""Trainium2 Bass kernel for CudaTensorProduct (e3nn-style COO tensor product).

Computation: out[b, o] = sum_k cb[k] * in1[b, idx1[k]] * in2[b, idx2[k]]
  in1/in2: (16384, 32) f32, out: (16384, 1024) f32, nnz=4528.

Strategy (per core, pure data-parallel over batch, 2048 rows/core):
  The l-structure (ls1=ls2=[0,1,2,3]x2) factorizes: permute in1 columns into
  4 "i-sets" of 8 ({l1=0,3} and {l1=1,2} per copy); in2 columns split into
  2 "j-sets" of 16 (the two copies). Every (l1,l2,l3) coupling then lives in
  exactly one of the 8 pair-tiles q=(a,b) = iset_a x jset_b, each with
  exactly 128 (i,j) pairs AND exactly 128 output rows -> the coefficient
  matrix W is block-diagonal over q with 128x128 blocks.

  Per core (inputs host-pre-transposed to in12T (64, 2048) bf16):
    R1_a = E1a.T @ in12T   (PE; 8 i-rows each replicated 16x) -- shared by
                            both j-sets; 4 matmuls per 512-chunk total
    R2_b = E2b.T @ in12T   (PE; 16 j-rows tiled 8x) -- shared by 4 i-sets
    cast R PSUM->SBUF bf16 (ACT/GPSIMD)
    U_q  = R1_a * R2_b     (DVE scalar_tensor_tensor, all-bf16 SBUF = 4x mode)
    outT_q = W_q.T @ U_q   (PE, bf16, PSUM fp32 accum)
    cast out PSUM->SBUF bf16 (ACT/GPSIMD/DVE), DMA bf16 to HBM.

  14 weight loads + 56 matmuls of 512 cols per core; host un-permutes and
  upcasts the bf16 output to fp32 during the unshard (pure layout).
"""

import os
import sys
import numpy as np
import ml_dtypes

sys.path.insert(0, "/opt/trn_rl_repo")

import concourse.bass as bass
import concourse.mybir as mybir
import concourse.tile as tile
from concourse import bacc
from concourse.bass_utils import run_bass_kernel_spmd

N_CORES = 8
B = 16384
BC = B // N_CORES          # 2048 batch rows per core
D1 = 32
D2 = 32
DOUT = D1 * D2             # 1024
NQ = 8                     # pair-tiles (4 i-sets x 2 j-sets)
CHUNK = 512                # batch columns per matmul
NCHUNK = BC // CHUNK       # 4
F32 = mybir.dt.float32
BF16 = mybir.dt.bfloat16
MULT = mybir.AluOpType.mult

LS = [0, 1, 2, 3, 0, 1, 2, 3]


# ----------------------------------------------------------------------------
# Host-side table preprocessing
# ----------------------------------------------------------------------------

def _build_tables(idx1, idx2, out_idx, cb_vals):
    """Build the factorized layout.

    Returns (iperm, e12, w, rows_map):
      iperm: (32,) permutation of in1 columns (iset-major).
      e12: (64, 6*128) bf16 -- E1a at cols a*128.. (a=0..3), E2b at
           cols (4+b)*128.. (b=0,1); rows index in12T partitions.
      w:   (128, 8*128) bf16 -- w[p, q*128+m] = coefficient for pair p
           (p = i_local*16 + j_local) into scratch out row q*128+m.
      rows_map: (1024,) int -- scratch row -> real out column.
    """
    idx1 = np.asarray(idx1, np.int64)
    idx2 = np.asarray(idx2, np.int64)
    out_idx = np.asarray(out_idx, np.int64)
    cb = np.asarray(cb_vals, np.float64)

    offs, blocks = 0, []
    for l in LS:
        blocks.append(list(range(offs, offs + 2 * l + 1)))
        offs += 2 * l + 1
    isets = [blocks[0] + blocks[3], blocks[1] + blocks[2],
             blocks[4] + blocks[7], blocks[5] + blocks[6]]
    jsets = [list(range(16)), list(range(16, 32))]
    imap = {c: (a, il) for a, s in enumerate(isets) for il, c in enumerate(s)}
    jmap = {c: (b, jl) for b, s in enumerate(jsets) for jl, c in enumerate(s)}

    out_q = {}
    for k in range(len(cb)):
        a, _ = imap[int(idx1[k])]
        b, _ = jmap[int(idx2[k])]
        q = a * 2 + b
        o = int(out_idx[k])
        assert out_q.setdefault(o, q) == q, "coupling crosses pair-tiles"
    rows_map = np.zeros(NQ * 128, np.int64)
    out_local = {}
    for q in range(NQ):
        outs = sorted(o for o, qq in out_q.items() if qq == q)
        assert len(outs) == 128, (q, len(outs))
        for m, o in enumerate(outs):
            out_local[o] = m
            rows_map[q * 128 + m] = o

    e12 = np.zeros((64, 6 * 128), np.float32)
    for a in range(4):
        for p in range(128):
            e12[a * 8 + p // 16, a * 128 + p] = 1.0
    for b in range(2):
        for p in range(128):
            e12[32 + b * 16 + p % 16, (4 + b) * 128 + p] = 1.0

    w = np.zeros((128, NQ * 128), np.float64)
    for k in range(len(cb)):
        a, il = imap[int(idx1[k])]
        b, jl = jmap[int(idx2[k])]
        q = a * 2 + b
        p = il * 16 + jl
        m = out_local[int(out_idx[k])]
        w[p, q * 128 + m] += cb[k]

    iperm = np.concatenate([np.asarray(s) for s in isets])
    bf = ml_dtypes.bfloat16
    return iperm, e12.astype(bf), w.astype(np.float32).astype(bf), rows_map


# ----------------------------------------------------------------------------
# Device program
# ----------------------------------------------------------------------------

def _build_bass():
    nc = bacc.Bacc("TRN2", target_bir_lowering=False)

    in12h = nc.dram_tensor("in12h", [64, BC], BF16, kind="ExternalInput")
    e12 = nc.dram_tensor("e12", [64, 6 * 128], BF16, kind="ExternalInput")
    wgt = nc.dram_tensor("wgt", [128, NQ * 128], BF16, kind="ExternalInput")
    outT = nc.dram_tensor("outT", [NQ * 128, BC], BF16, kind="ExternalOutput")

    # R slots 0-3 = R1 (isets), 4-5 = R2 (jsets).
    # Emission program: 16 R-matmuls up front (PE p-state ramp), then mains
    # interleaved with the last two R slots so the PE never drains. GPSIMD
    # (slow, SBUF-only) gets the two muls consumed last, emitted early.
    # 'R' = slot matmuls+casts, 'M' = DVE mul, 'MG' = GPSIMD mul,
    # 'Q' = main matmuls + out casts + DMA for pair-tile (a, b).
    program = [
        ('R', 0), ('R', 4), ('R', 1), ('MG', (1, 0)), ('R', 5), ('MG', (0, 1)),
        ('M', (0, 0)), ('M', (1, 1)),
        ('Q', (0, 0)), ('R', 2), ('M', (2, 0)), ('M', (2, 1)),
        ('Q', (1, 1)), ('R', 3), ('M', (3, 1)), ('M', (3, 0)),
        ('Q', (2, 0)), ('Q', (3, 1)), ('Q', (2, 1)), ('Q', (3, 0)),
        ('Q', (1, 0)), ('Q', (0, 1)),
    ]
    gps_muls = {(0, 1), (1, 0)}
    # cast engine per op: R slots 0,4,1,5 alternate ACT/DVE; slots 2,3 all
    # ACT (keeps DVE free for muls); out casts ~12 ACT / 4 DVE.
    r_cast_plan = {0: 'av', 4: 'av', 1: 'av', 5: 'av', 2: 'aa', 3: 'aa'}
    o_cast_seq = 'aavaaavaaavaaava'

    with tile.TileContext(nc) as tc:
        with (
            tc.tile_pool(name="const", bufs=1) as const_pool,
            tc.tile_pool(name="work", bufs=1) as work_pool,
            tc.tile_pool(name="ps_r", bufs=2, space="PSUM") as ps_r_pool,
            tc.tile_pool(name="ps_o", bufs=2, space="PSUM") as ps_o_pool,
        ):
            e_sb = const_pool.tile([64, 6 * 128], BF16)
            nc.sync.dma_start(out=e_sb[:], in_=e12.ap())
            x_sb = work_pool.tile([64, BC], BF16)
            # split the input DMA so the first R-matmuls start sooner
            nc.sync.dma_start(out=x_sb[:, : BC // 2], in_=in12h.ap()[:, : BC // 2])
            nc.sync.dma_start(out=x_sb[:, BC // 2 :], in_=in12h.ap()[:, BC // 2 :])
            w_sb = const_pool.tile([128, NQ * 128], BF16)
            nc.sync.dma_start(out=w_sb[:], in_=wgt.ap())

            r_sb = work_pool.tile([128, 6 * BC], BF16)
            u_sb = work_pool.tile([128, NQ * BC], BF16)
            osb = work_pool.tile([128, NQ * BC], BF16)

            def emit_cast(code, dst, ps):
                if code == 'a':
                    nc.scalar.copy(out=dst, in_=ps[:])
                else:
                    nc.vector.tensor_copy(dst, ps[:])

            oci = [0]
            for kind, arg in program:
                if kind == 'R':
                    slot = arg
                    for h in range(2):  # halves of BC: 2 chunks each
                        ps = ps_r_pool.tile([128, 2 * CHUNK], F32)
                        for ci in range(2):
                            c = h * 2 + ci
                            nc.tensor.matmul(
                                ps[:, ci * CHUNK : (ci + 1) * CHUNK],
                                lhsT=e_sb[:, slot * 128 : (slot + 1) * 128],
                                rhs=x_sb[:, c * CHUNK : (c + 1) * CHUNK],
                                start=True,
                                stop=True,
                            )
                        emit_cast(
                            r_cast_plan[slot][h],
                            r_sb[:, slot * BC + h * 1024 : slot * BC + (h + 1) * 1024],
                            ps,
                        )
                elif kind in ('M', 'MG'):
                    a, b = arg
                    q = a * 2 + b
                    out_ap = u_sb[:, q * BC : (q + 1) * BC]
                    in0 = r_sb[:, a * BC : (a + 1) * BC]
                    in1 = r_sb[:, (4 + b) * BC : (5 + b) * BC]
                    if kind == 'MG':
                        # Pool engine: only plain TensorTensor is supported
                        nc.gpsimd.tensor_mul(out_ap, in0, in1)
                    else:
                        nc.vector.tensor_mul(out_ap, in0, in1)
                else:  # 'Q': mains + out casts + per-half DMA
                    a, b = arg
                    q = a * 2 + b
                    for h in range(2):
                        ps = ps_o_pool.tile([128, 2 * CHUNK], F32)
                        for ci in range(2):
                            c = h * 2 + ci
                            nc.tensor.matmul(
                                ps[:, ci * CHUNK : (ci + 1) * CHUNK],
                                lhsT=w_sb[:, q * 128 : (q + 1) * 128],
                                rhs=u_sb[:, q * BC + c * CHUNK : q * BC + (c + 1) * CHUNK],
                                start=True,
                                stop=True,
                            )
                        emit_cast(
                            o_cast_seq[oci[0] % len(o_cast_seq)],
                            osb[:, q * BC + h * 1024 : q * BC + (h + 1) * 1024],
                            ps,
                        )
                        oci[0] += 1
                        nc.sync.dma_start(
                            out=outT.ap()[
                                q * 128 : (q + 1) * 128, h * 1024 : (h + 1) * 1024
                            ],
                            in_=osb[:, q * BC + h * 1024 : q * BC + (h + 1) * 1024],
                        )
    nc.compile()
    return nc


# ----------------------------------------------------------------------------
# Entry point
# ----------------------------------------------------------------------------

_CACHE = {}


def kernel(in1, in2, cb_vals, idx1, idx2, out_idx):
    in1 = np.ascontiguousarray(np.asarray(in1, np.float32))
    in2 = np.ascontiguousarray(np.asarray(in2, np.float32))

    key = (
        np.asarray(idx1).tobytes(),
        np.asarray(idx2).tobytes(),
        np.asarray(out_idx).tobytes(),
        np.asarray(cb_vals).tobytes(),
    )
    kh = hash(key)
    if kh not in _CACHE:
        iperm, e12, w, rows_map = _build_tables(idx1, idx2, out_idx, cb_vals)
        nc = _build_bass()
        _CACHE[kh] = (nc, iperm, e12, w, rows_map)
    nc, iperm, e12, w, rows_map = _CACHE[kh]

    bf = ml_dtypes.bfloat16
    in1p = in1[:, iperm]
    in_maps = []
    for core in range(N_CORES):
        sl = slice(core * BC, (core + 1) * BC)
        in12h = np.ascontiguousarray(
            np.concatenate([in1p[sl], in2[sl]], axis=1).T.astype(bf)
        )  # (64, BC)
        in_maps.append({"in12h": in12h, "e12": e12, "wgt": w})

    trace = bool(int(os.environ.get("KERNEL_TRACE", "0")))
    res = run_bass_kernel_spmd(
        nc, in_maps, core_ids=list(range(N_CORES)), trace=trace
    )
    kernel.last_results = res

    out = np.empty((B, DOUT), np.float32)
    for core in range(N_CORES):
        shard = res.results[core]["outT"]  # (1024, BC) bf16 scratch layout
        out[core * BC : (core + 1) * BC][:, rows_map] = (
            np.asarray(shard).astype(np.float32).T
        )
    return out
